# revision 1
# baseline (speedup 1.0000x reference)
"""LoRA QKV kernel for TRN2, 8 NeuronCores, data-parallel over rows.

y = x @ W_qkv^T + b_qkv ; q += (x a_q^T) b_q^T /16 ; v += (x a_v^T) b_v^T /16

Strategy:
 - shard the 4*4096=16384 rows across 8 cores (2048 rows each), replicate weights
 - host-side: transpose x shard to [K=1024, M=2048] and split all matmul operands
   into bf16 hi/lo pairs; f32 product reconstructed as xh@wh + xh@wl + xl@wh
   (error ~2^-18, PE runs at full bf16 rate)
 - LoRA: z = x@a^T computed once per chunk into PSUM (a replicated 3x in 32-row
   groups), split z into bf16 hi/lo on device, then accumulated straight into the
   main QKV PSUM banks via K=32 row-group-packed matmuls (b matrices carry /16)
 - bias added during the PSUM->SBUF copy (DVE tensor_add with host-replicated bias)
"""
import numpy as np
import ml_dtypes

import concourse.bass as bass
import concourse.mybir as mybir
import concourse.tile as tile
from concourse import bass_utils

D = 1024          # d_model (K)
NO = 3072         # 3 * nh_kd (N)
R = 16            # LoRA rank
SCALING = 1.0 / 16.0
N_CORES = 8
ROWS = 4 * 4096
M_CORE = ROWS // N_CORES      # 2048
KT = D // 128                 # 8 k-tiles
M_CHUNK = 512                 # rows per x-load chunk
N_TILE = 512                  # psum free dim
BF16 = ml_dtypes.bfloat16

TRACE = False
_CACHE = {}


def _split(a):
    hi = a.astype(BF16)
    lo = (a - hi.astype(np.float32)).astype(BF16)
    return np.ascontiguousarray(hi), np.ascontiguousarray(lo)


def _build_nc():
    nc = bass.Bass()
    dt = mybir.dt
    xh_d = nc.dram_tensor("xh", (D, M_CORE), dt.bfloat16, kind="ExternalInput")
    xl_d = nc.dram_tensor("xl", (D, M_CORE), dt.bfloat16, kind="ExternalInput")
    wh_d = nc.dram_tensor("wh", (D, NO), dt.bfloat16, kind="ExternalInput")
    wl_d = nc.dram_tensor("wl", (D, NO), dt.bfloat16, kind="ExternalInput")
    ah_d = nc.dram_tensor("ah", (D, 32), dt.bfloat16, kind="ExternalInput")
    al_d = nc.dram_tensor("al", (D, 32), dt.bfloat16, kind="ExternalInput")
    bph_d = nc.dram_tensor("bph", (32, NO), dt.bfloat16, kind="ExternalInput")
    bpl_d = nc.dram_tensor("bpl", (32, NO), dt.bfloat16, kind="ExternalInput")
    bias_d = nc.dram_tensor("bias", (128, NO), dt.float32, kind="ExternalInput")
    out_d = nc.dram_tensor("out", (M_CORE, NO), dt.float32, kind="ExternalOutput")

    n_chunks = M_CORE // M_CHUNK
    msubs = M_CHUNK // 128
    n_tiles = NO // N_TILE
    # n-tile index -> lora region (cols of bp to use), or None for k region
    lora_region = {0: True, 1: True, 2: False, 3: False, 4: True, 5: True}

    with tile.TileContext(nc) as tc:
        with tc.tile_pool(name="wres", bufs=1) as wres, \
             tc.tile_pool(name="xin", bufs=2) as xin, \
             tc.tile_pool(name="zbuf", bufs=2) as zbuf, \
             tc.tile_pool(name="obuf", bufs=4) as obuf, \
             tc.tile_pool(name="psz", bufs=2, space="PSUM") as psz, \
             tc.tile_pool(name="psm", bufs=4, space="PSUM") as psm:
            # resident weights
            wh_sb = wres.tile([128, KT, NO], dt.bfloat16, tag="wh")
            wl_sb = wres.tile([128, KT, NO], dt.bfloat16, tag="wl")
            nc.sync.dma_start(wh_sb[:], wh_d[:, :].rearrange("(kt p) n -> p kt n", p=128))
            nc.sync.dma_start(wl_sb[:], wl_d[:, :].rearrange("(kt p) n -> p kt n", p=128))
            ah_sb = wres.tile([128, KT, 32], dt.bfloat16, tag="ah")
            al_sb = wres.tile([128, KT, 32], dt.bfloat16, tag="al")
            nc.sync.dma_start(ah_sb[:], ah_d[:, :].rearrange("(kt p) n -> p kt n", p=128))
            nc.sync.dma_start(al_sb[:], al_d[:, :].rearrange("(kt p) n -> p kt n", p=128))
            bph_sb = wres.tile([32, NO], dt.bfloat16, tag="bph")
            nc.sync.dma_start(bph_sb[:], bph_d[:, :])
            bpl_sb = wres.tile([32, NO], dt.bfloat16, tag="bpl")
            nc.sync.dma_start(bpl_sb[:], bpl_d[:, :])
            bias_sb = wres.tile([128, NO], dt.float32, tag="bias")
            nc.sync.dma_start(bias_sb[:], bias_d[:, :])
            scr = wres.tile([1, 16], dt.float32, tag="scr")
            # absorb the bias-DMA wait once so later tensor_adds carry none
            nc.vector.tensor_copy(scr[0:1, 0:1], bias_sb[0:1, 0:1])

            for ch in range(n_chunks):
                m0 = ch * M_CHUNK
                xh_sb = xin.tile([128, KT, M_CHUNK], dt.bfloat16, tag="xh")
                xl_sb = xin.tile([128, KT, M_CHUNK], dt.bfloat16, tag="xl")
                nc.sync.dma_start(
                    xh_sb[:], xh_d[:, m0:m0 + M_CHUNK].rearrange("(kt p) m -> p kt m", p=128))
                nc.sync.dma_start(
                    xl_sb[:], xl_d[:, m0:m0 + M_CHUNK].rearrange("(kt p) m -> p kt m", p=128))

                # ---- LoRA z = x @ a^T (a replicated in three 32-row groups) ----
                pz = psz.tile([32, M_CHUNK], dt.float32, tag="pz")
                terms = ((ah_sb, xh_sb), (al_sb, xh_sb), (ah_sb, xl_sb))
                for ti, (aa, xx) in enumerate(terms):
                    for k in range(KT):
                        nc.tensor.matmul(
                            pz[:, :], aa[:, k, :], xx[:, k, :],
                            start=(ti == 0 and k == 0),
                            stop=(ti == 2 and k == KT - 1))
                # split z into bf16 hi/lo at partition base 0
                zth = zbuf.tile([32, M_CHUNK], dt.bfloat16, tag="zth")
                ztl = zbuf.tile([32, M_CHUNK], dt.bfloat16, tag="ztl")
                nc.vector.tensor_copy(zth[:, :], pz[:, :])
                nc.vector.tensor_sub(ztl[:, :], pz[:, :], zth[:, :])

                # ---- main QKV + fused LoRA accumulation ----
                for ms in range(msubs):
                    mm0 = ms * 128
                    for n in range(n_tiles):
                        nn0 = n * N_TILE
                        pm = psm.tile([128, N_TILE], dt.float32, tag="pm")
                        has_lora = lora_region[n]
                        wterms = ((xh_sb, wh_sb), (xh_sb, wl_sb), (xl_sb, wh_sb))
                        for ti, (xx, ww) in enumerate(wterms):
                            for k in range(KT):
                                nc.tensor.matmul(
                                    pm[:, :],
                                    xx[:, k, mm0:mm0 + 128],
                                    ww[:, k, nn0:nn0 + N_TILE],
                                    start=(ti == 0 and k == 0),
                                    stop=(not has_lora and ti == 2 and k == KT - 1))
                        if has_lora:
                            lterms = ((zth, bph_sb), (ztl, bph_sb), (zth, bpl_sb))
                            for g, (zz, bb) in enumerate(lterms):
                                nc.tensor.matmul(
                                    pm[:, :],
                                    zz[:, mm0:mm0 + 128],
                                    bb[:, nn0:nn0 + N_TILE],
                                    start=False, stop=(g == 2))
                        ob = obuf.tile([128, N_TILE], dt.float32, tag="ob")
                        # wait-absorbers: WAR on ob slot, RAW on pm (1 wait each)
                        nc.vector.memset(ob[0:1, 0:1], 0.0)
                        nc.vector.tensor_copy(scr[0:1, 1:2], pm[0:1, 0:1])
                        nc.vector.tensor_add(ob[:, :], pm[:, :], bias_sb[:, nn0:nn0 + N_TILE])
                        nc.sync.dma_start(
                            out_d[m0 + mm0:m0 + mm0 + 128, nn0:nn0 + N_TILE], ob[:, :])
    _split_multi_waits(nc)
    return nc


def _split_multi_waits(nc):
    """This walrus build fuses at most one sync-wait per instruction; hoist
    extras onto engine-matched NoOps inserted immediately before."""
    dt = mybir.dt
    uid = [0]
    for fn in nc.m.functions:
        for blk in fn.blocks:
            out = []
            for ins in blk.instructions:
                si = ins.sync_info
                waits = list(si.on_wait) if si is not None and si.on_wait else []
                if len(waits) > 1:
                    for w in waits[:-1]:
                        nop = mybir.InstNoOp(name=f"waitnop_{uid[0]}", ins=[], outs=[])
                        uid[0] += 1
                        nop.engine = ins.engine
                        nop.sync_info = mybir.SyncInfo(on_wait=[w], on_update=[])
                        out.append(nop)
                    ins.sync_info = mybir.SyncInfo(
                        on_wait=[waits[-1]],
                        on_update=list(si.on_update) if si.on_update else [])
                out.append(ins)
            blk.instructions = out


def _prep_shared(w_qkv, b_qkv, a_q, b_q, a_v, b_v):
    wT = np.ascontiguousarray(w_qkv.T.astype(np.float32))       # (1024, 3072)
    wh, wl = _split(wT)
    A = np.zeros((D, 32), np.float32)
    A[:, 0:16] = a_q.T
    A[:, 16:32] = a_v.T
    ah, al = _split(A)
    # b groups: 0 -> hi, 1 -> hi (pairs z_lo), 2 -> lo; q rows 0:16, v rows 16:32
    Bq = (b_q.T * SCALING).astype(np.float32)                   # (16, 1024)
    Bv = (b_v.T * SCALING).astype(np.float32)
    Bfull = np.zeros((32, NO), np.float32)
    Bfull[0:16, 0:D] = Bq
    Bfull[16:32, 2 * D:3 * D] = Bv
    Bh, Bl = _split(Bfull)
    bias = np.ascontiguousarray(
        np.broadcast_to(b_qkv.astype(np.float32), (128, NO)))
    return wh, wl, ah, al, Bh, Bl, bias


def kernel(x, w_qkv, b_qkv, a_q, b_q, a_v, b_v):
    """Data-parallel over the 8 NeuronCores via jax shard_map (rows split 8x,
    weights replicated). The Bass/Tile path (_build_nc) compiles and runs but
    still has a correctness bug, so the sharded-XLA path is used for output."""
    import jax
    import jax.numpy as jnp
    from jax.sharding import Mesh, PartitionSpec as P
    from jax.experimental.shard_map import shard_map
    devs = jax.devices()[:N_CORES]
    mesh = Mesh(np.asarray(devs), ("c",))
    hp = jax.lax.Precision.HIGHEST

    def per_core(xc, wT, b, aqT, bqT, avT, bvT):
        y = jnp.dot(xc, wT, precision=hp) + b
        zq = jnp.dot(jnp.dot(xc, aqT, precision=hp), bqT, precision=hp)
        zv = jnp.dot(jnp.dot(xc, avT, precision=hp), bvT, precision=hp)
        y = y.at[:, 0:D].add(zq * SCALING)
        y = y.at[:, 2 * D:3 * D].add(zv * SCALING)
        return y

    f = shard_map(per_core, mesh=mesh,
                  in_specs=(P("c"), P(), P(), P(), P(), P(), P()),
                  out_specs=P("c"), check_rep=False)
    X = np.asarray(x, np.float32).reshape(ROWS, D)
    out = jax.jit(f)(X, np.asarray(w_qkv).T.copy(), np.asarray(b_qkv),
                     np.asarray(a_q).T.copy(), np.asarray(b_q).T.copy(),
                     np.asarray(a_v).T.copy(), np.asarray(b_v).T.copy())
    return np.asarray(out).reshape(4, 4096, NO)


def kernel_bass(x, w_qkv, b_qkv, a_q, b_q, a_v, b_v):
    x = np.asarray(x, np.float32)
    wh, wl, ah, al, bph, bpl, bias = _prep_shared(
        np.asarray(w_qkv), np.asarray(b_qkv), np.asarray(a_q),
        np.asarray(b_q), np.asarray(a_v), np.asarray(b_v))
    X = x.reshape(ROWS, D)
    in_maps = []
    for c in range(N_CORES):
        xT = np.ascontiguousarray(X[c * M_CORE:(c + 1) * M_CORE].T)
        xh, xl = _split(xT)
        in_maps.append({"xh": xh, "xl": xl, "wh": wh, "wl": wl,
                        "ah": ah, "al": al, "bph": bph, "bpl": bpl, "bias": bias})
    if "nc" not in _CACHE:
        _CACHE["nc"] = _build_nc()
    nc = _CACHE["nc"]
    res = bass_utils.run_bass_kernel_spmd(
        nc, in_maps, core_ids=list(range(N_CORES)), trace=TRACE)
    if TRACE:
        _CACHE["last_exec_time_ns"] = res.exec_time_ns
    out = np.concatenate([res.results[c]["out"] for c in range(N_CORES)], axis=0)
    return out.reshape(4, 4096, NO)



# revision 2
# speedup vs baseline: 14173.5484x; 14173.5484x over previous
"""LoRA QKV kernel for TRN2, 8 NeuronCores, data-parallel over rows.

y = x @ W_qkv^T + b_qkv ; q += (x a_q^T) b_q^T /16 ; v += (x a_v^T) b_v^T /16

Strategy:
 - shard the 4*4096=16384 rows across 8 cores (2048 rows each), replicate weights
 - host-side: transpose x shard to [K=1024, M=2048] and split all matmul operands
   into bf16 hi/lo pairs; f32 product reconstructed as xh@wh + xh@wl + xl@wh
   (error ~2^-18, PE runs at full bf16 rate)
 - LoRA is small relative to the rel-err budget (z@b*scaling ~ 5e-3 of output
   scale), so single bf16 term everywhere: z = xh@ah into PSUM, cast to bf16,
   then one K=32 row-group-packed matmul (b carries /16) accumulated straight
   into the main QKV PSUM banks
 - bias added during the PSUM->SBUF copy (DVE tensor_add with host-replicated bias)
 - DMA order: small tensors + first x chunk first, then weights interleaved
   per 512-col n-tile so the first matmul group starts as early as possible
"""
import numpy as np
import ml_dtypes

import concourse.bass as bass
import concourse.mybir as mybir
import concourse.tile as tile
from concourse import bass_utils

D = 1024          # d_model (K)
NO = 3072         # 3 * nh_kd (N)
R = 16            # LoRA rank
SCALING = 1.0 / 16.0
N_CORES = 8
ROWS = 4 * 4096
M_CORE = ROWS // N_CORES      # 2048
KT = D // 128                 # 8 k-tiles
M_CHUNK = 512                 # rows per x-load chunk
N_TILE = 512                  # psum free dim
BF16 = ml_dtypes.bfloat16

TRACE = False
_CACHE = {}


def _split(a):
    hi = a.astype(BF16)
    lo = (a - hi.astype(np.float32)).astype(BF16)
    return np.ascontiguousarray(hi), np.ascontiguousarray(lo)


def _build_nc():
    nc = bass.Bass()
    dt = mybir.dt
    xh_d = nc.dram_tensor("xh", (D, M_CORE), dt.bfloat16, kind="ExternalInput")
    xl_d = nc.dram_tensor("xl", (D, M_CORE), dt.bfloat16, kind="ExternalInput")
    wh_d = nc.dram_tensor("wh", (D, NO), dt.bfloat16, kind="ExternalInput")
    wl_d = nc.dram_tensor("wl", (D, NO), dt.bfloat16, kind="ExternalInput")
    ah_d = nc.dram_tensor("ah", (D, 32), dt.bfloat16, kind="ExternalInput")
    bph_d = nc.dram_tensor("bph", (32, NO), dt.bfloat16, kind="ExternalInput")
    bias_d = nc.dram_tensor("bias", (128, NO), dt.float32, kind="ExternalInput")
    out_d = nc.dram_tensor("out", (M_CORE, NO), dt.float32, kind="ExternalOutput")

    n_chunks = M_CORE // M_CHUNK
    msubs = M_CHUNK // 128
    n_tiles = NO // N_TILE
    # n-tile index -> lora region (cols of bp to use), or None for k region
    lora_region = {0: True, 1: True, 2: False, 3: False, 4: True, 5: True}

    with tile.TileContext(nc) as tc:
        with tc.tile_pool(name="wres", bufs=1) as wres, \
             tc.tile_pool(name="xin", bufs=2) as xin, \
             tc.tile_pool(name="zbuf", bufs=2) as zbuf, \
             tc.tile_pool(name="obuf", bufs=4) as obuf, \
             tc.tile_pool(name="psz", bufs=2, space="PSUM") as psz, \
             tc.tile_pool(name="psm", bufs=4, space="PSUM") as psm:
            # small resident tensors first so early compute isn't stuck
            # behind the 12.6MB weight stream
            ah_sb = wres.tile([128, KT, 32], dt.bfloat16, tag="ah")
            nc.sync.dma_start(ah_sb[:], ah_d[:, :].rearrange("(kt p) n -> p kt n", p=128))
            bph_sb = wres.tile([32, NO], dt.bfloat16, tag="bph")
            nc.sync.dma_start(bph_sb[:], bph_d[:, :])
            bias_sb = wres.tile([128, NO], dt.float32, tag="bias")
            nc.sync.dma_start(bias_sb[:], bias_d[:, :])
            scr = wres.tile([1, 16], dt.float32, tag="scr")
            # absorb the bias-DMA wait once so later tensor_adds carry none
            nc.vector.tensor_copy(scr[0:1, 0:1], bias_sb[0:1, 0:1])

            # resident weights, one DMA per 512-col n-tile, hi/lo interleaved
            # so matmul group (chunk0, ms0, n) unblocks in n order
            wh_sb = wres.tile([128, KT, NO], dt.bfloat16, tag="wh")
            wl_sb = wres.tile([128, KT, NO], dt.bfloat16, tag="wl")
            for n in range(n_tiles):
                nn0 = n * N_TILE
                nc.sync.dma_start(
                    wh_sb[:, :, nn0:nn0 + N_TILE],
                    wh_d[:, nn0:nn0 + N_TILE].rearrange("(kt p) n -> p kt n", p=128))
                nc.sync.dma_start(
                    wl_sb[:, :, nn0:nn0 + N_TILE],
                    wl_d[:, nn0:nn0 + N_TILE].rearrange("(kt p) n -> p kt n", p=128))

            for ch in range(n_chunks):
                m0 = ch * M_CHUNK
                xh_sb = xin.tile([128, KT, M_CHUNK], dt.bfloat16, tag="xh")
                xl_sb = xin.tile([128, KT, M_CHUNK], dt.bfloat16, tag="xl")
                nc.sync.dma_start(
                    xh_sb[:], xh_d[:, m0:m0 + M_CHUNK].rearrange("(kt p) m -> p kt m", p=128))
                nc.sync.dma_start(
                    xl_sb[:], xl_d[:, m0:m0 + M_CHUNK].rearrange("(kt p) m -> p kt m", p=128))

                # ---- LoRA z = x @ a^T (a packed as q rows 0:16, v rows 16:32) ----
                pz = psz.tile([32, M_CHUNK], dt.float32, tag="pz")
                for k in range(KT):
                    nc.tensor.matmul(
                        pz[:, :], ah_sb[:, k, :], xh_sb[:, k, :],
                        start=(k == 0), stop=(k == KT - 1))
                zth = zbuf.tile([32, M_CHUNK], dt.bfloat16, tag="zth")
                nc.vector.tensor_copy(zth[:, :], pz[:, :])

                # ---- main QKV + fused LoRA accumulation ----
                for ms in range(msubs):
                    mm0 = ms * 128
                    for n in range(n_tiles):
                        nn0 = n * N_TILE
                        pm = psm.tile([128, N_TILE], dt.float32, tag="pm")
                        has_lora = lora_region[n]
                        wterms = ((xh_sb, wh_sb), (xh_sb, wl_sb), (xl_sb, wh_sb))
                        for ti, (xx, ww) in enumerate(wterms):
                            for k in range(KT):
                                nc.tensor.matmul(
                                    pm[:, :],
                                    xx[:, k, mm0:mm0 + 128],
                                    ww[:, k, nn0:nn0 + N_TILE],
                                    start=(ti == 0 and k == 0),
                                    stop=(not has_lora and ti == 2 and k == KT - 1))
                        if has_lora:
                            nc.tensor.matmul(
                                pm[:, :],
                                zth[:, mm0:mm0 + 128],
                                bph_sb[:, nn0:nn0 + N_TILE],
                                start=False, stop=True)
                        ob = obuf.tile([128, N_TILE], dt.float32, tag="ob")
                        # wait-absorbers: WAR on ob slot, RAW on pm (1 wait each)
                        nc.vector.memset(ob[0:1, 0:1], 0.0)
                        nc.vector.tensor_copy(scr[0:1, 1:2], pm[0:1, 0:1])
                        nc.vector.tensor_add(ob[:, :], pm[:, :], bias_sb[:, nn0:nn0 + N_TILE])
                        nc.sync.dma_start(
                            out_d[m0 + mm0:m0 + mm0 + 128, nn0:nn0 + N_TILE], ob[:, :])
    _split_multi_waits(nc)
    return nc


def _split_multi_waits(nc):
    """This walrus build fuses at most one sync-wait per instruction; hoist
    extras onto engine-matched NoOps inserted immediately before."""
    dt = mybir.dt
    uid = [0]
    for fn in nc.m.functions:
        for blk in fn.blocks:
            out = []
            for ins in blk.instructions:
                si = ins.sync_info
                waits = list(si.on_wait) if si is not None and si.on_wait else []
                if len(waits) > 1:
                    for w in waits[:-1]:
                        nop = mybir.InstNoOp(name=f"waitnop_{uid[0]}", ins=[], outs=[])
                        uid[0] += 1
                        nop.engine = ins.engine
                        nop.sync_info = mybir.SyncInfo(on_wait=[w], on_update=[])
                        out.append(nop)
                    ins.sync_info = mybir.SyncInfo(
                        on_wait=[waits[-1]],
                        on_update=list(si.on_update) if si.on_update else [])
                out.append(ins)
            blk.instructions = out


def _prep_shared(w_qkv, b_qkv, a_q, b_q, a_v, b_v):
    wT = np.ascontiguousarray(w_qkv.T.astype(np.float32))       # (1024, 3072)
    wh, wl = _split(wT)
    A = np.zeros((D, 32), np.float32)
    A[:, 0:16] = a_q.T
    A[:, 16:32] = a_v.T
    ah = np.ascontiguousarray(A.astype(BF16))
    Bq = (b_q.T * SCALING).astype(np.float32)                   # (16, 1024)
    Bv = (b_v.T * SCALING).astype(np.float32)
    Bfull = np.zeros((32, NO), np.float32)
    Bfull[0:16, 0:D] = Bq
    Bfull[16:32, 2 * D:3 * D] = Bv
    bph = np.ascontiguousarray(Bfull.astype(BF16))
    bias = np.ascontiguousarray(
        np.broadcast_to(b_qkv.astype(np.float32), (128, NO)))
    return wh, wl, ah, bph, bias


def kernel(x, w_qkv, b_qkv, a_q, b_q, a_v, b_v):
    x = np.asarray(x, np.float32)
    wh, wl, ah, bph, bias = _prep_shared(
        np.asarray(w_qkv), np.asarray(b_qkv), np.asarray(a_q),
        np.asarray(b_q), np.asarray(a_v), np.asarray(b_v))
    X = x.reshape(ROWS, D)
    in_maps = []
    for c in range(N_CORES):
        xT = np.ascontiguousarray(X[c * M_CORE:(c + 1) * M_CORE].T)
        xh, xl = _split(xT)
        in_maps.append({"xh": xh, "xl": xl, "wh": wh, "wl": wl,
                        "ah": ah, "bph": bph, "bias": bias})
    if "nc" not in _CACHE:
        _CACHE["nc"] = _build_nc()
    nc = _CACHE["nc"]
    res = bass_utils.run_bass_kernel_spmd(
        nc, in_maps, core_ids=list(range(N_CORES)), trace=TRACE)
    if TRACE:
        _CACHE["last_exec_time_ns"] = res.exec_time_ns
        _CACHE["last_result"] = res
    out = np.concatenate([res.results[c]["out"] for c in range(N_CORES)], axis=0)
    return out.reshape(4, 4096, NO)
